# revision 31
# baseline (speedup 1.0000x reference)
"""IsoMaxPlus first-part kernel for Trainium2 (8 NeuronCores, SPMD).

Math (per point n, prototype k):
    c[n,k] = (x_n . p_hat_k) / ||x_n||          (cosine sim)
    out[n,k] = -|s| * sqrt(2 - 2 c[n,k])

The device computes u = |s|*sqrt(2-2c) in bf16; the host negates during
the bf16 -> f32 upcast of the gather (a sqrt's sign cannot be flipped
on-device without a whole extra engine pass).

Per macro-tile of NF=1024 points (per core: 2 of 16 batches):
    DMA  xt [128, 2, NF] f32r                   (C=256 split in 2 chunks)
    DVE  q1 = xt0^2 (bf16)     ACT  q2 = Square(xt1) (bf16)
    DVE  qs = q1 + q2 (bf16)                    (fold 256-chan ssq to 128)
    PE   G[19,NF]  = W.T @ x   (fp32r, 2 passes; W = p_hat)
         S[1,NF]   = ones.T @ qs (bf16, 1 pass; = ||x||^2)
    ACT  r1[1,NF]  = Sqrt(S)                    (PSUM extract, = ||x||)
    DMA  cm[128,8] <- r1                        (compact: 8/lane not 1024)
    DVE  ic = 1/cm                              (= 1/||x||, compact)
    DMA  rid[tile] <- ic                        (DRAM round-trip ...)
    DMA  ribc[19,NF] <- rid (stride-0 x19)      (... broadcast to 19 rows)
    DVE  t = G * ribc                           (= c)
    ACT  u = Sqrt(-2s^2 t + 2s^2) (bf16)        (= |s| sqrt(d2))
    DMA  out <- u

The tail (ribc/t/u/out) of tile m is emitted during tile m+1 so the
compact-chain latency never head-of-line-blocks any engine. PE work is
3 streaming passes/tile -- the engine floor under the ~50% power
throttle observed on these cores (PE never ramps past ~1.2 GHz).
"""

import numpy as np

B, C, H, W = 16, 256, 128, 256
K = 19
NCORES = 8
BPC = B // NCORES          # batches per core
HW = H * W                 # 32768 points per batch
NF = 1024                  # points per macro-tile
EPS = 1e-12


def _split_excess_waits(nc):
    """Walrus limits the sync-wait slots per ISA instruction (TensorTensor
    takes only 1, DMAs 2, ...). Hoist excess waits onto same-engine NoOps
    inserted right before the instruction — engines execute in order, so
    all waits still complete before the instruction runs."""
    import bass_rust
    import concourse.mybir as mybir

    limits = {}
    default_limit = 1
    skip = {"InstEventSemaphore", "InstNoOp", "InstCall",
            "InstUnconditionalBranch", "InstISA", "InstRegisterMove"}
    nseq = 0
    for fn in nc.m.functions:
        for blk in fn.blocks:
            new = []
            for I in blk.instructions:
                tn = type(I).__name__
                si = I.sync_info
                waits = list(si.on_wait) if si else []
                lim = limits.get(tn, default_limit)
                if tn in skip or len(waits) <= lim:
                    new.append(I)
                    continue
                keep = waits[-lim:]
                excess = waits[:-lim]
                for w in excess:
                    nop = mybir.InstNoOp(name=f"{I.name}-w{nseq}", ins=[], outs=[])
                    nseq += 1
                    nop.engine = I.engine
                    nop.sync_info = bass_rust.SyncInfo(on_wait=[w], on_update=[])
                    new.append(nop)
                I.sync_info = bass_rust.SyncInfo(
                    on_wait=keep, on_update=list(si.on_update) if si else []
                )
                new.append(I)
            blk.instructions = new
    return nc


def build_program(bpc=BPC, hw=HW, nf=NF, split_waits=True):
    from contextlib import ExitStack

    import concourse.bass as bass
    import concourse.mybir as mybir
    import concourse.tile as tile

    f32 = mybir.dt.float32
    f32r = mybir.dt.float32r
    bf16 = mybir.dt.bfloat16
    AF = mybir.ActivationFunctionType
    nsub = nf // 512
    nmacro = hw // nf
    ncmp = nf // 128           # compact free size
    ntiles = bpc * nmacro

    nc = bass.Bass()
    # features/wproto declared float32r (same f32 bits) so the fp32r
    # matmuls see fp32r-dtype producers without any conversion step
    feat = nc.declare_dram_parameter("features", [bpc, C, hw], f32r, isOutput=False)
    wp = nc.declare_dram_parameter("wproto", [128, 2, K], f32r, isOutput=False)
    sv = nc.declare_dram_parameter("svec", [K, 1], f32, isOutput=False)
    bv = nc.declare_dram_parameter("bvec", [K, 1], f32, isOutput=False)
    out = nc.declare_dram_parameter("out", [bpc, K, hw], bf16, isOutput=True)
    rid = nc.dram_tensor("ridscratch", (ntiles, nf), bf16, kind="Internal")

    with ExitStack() as ctx:
        tc = ctx.enter_context(tile.TileContext(nc))
        singles = ctx.enter_context(tc.tile_pool(name="singles", bufs=1))
        xpool = ctx.enter_context(tc.tile_pool(name="x", bufs=8))
        qpool = ctx.enter_context(tc.tile_pool(name="q", bufs=6))
        gpool = ctx.enter_context(tc.tile_pool(name="g", bufs=4, space="PSUM"))
        spool = ctx.enter_context(tc.tile_pool(name="s", bufs=2, space="PSUM"))
        cpool = ctx.enter_context(tc.tile_pool(name="c", bufs=10))
        r1pool = ctx.enter_context(tc.tile_pool(name="r1", bufs=6))
        bpool = ctx.enter_context(tc.tile_pool(name="bc", bufs=4))
        tpool = ctx.enter_context(tc.tile_pool(name="t", bufs=4))
        opool = ctx.enter_context(tc.tile_pool(name="o", bufs=4))

        w_s = singles.tile([128, 2, K], f32r)
        nc.sync.dma_start(out=w_s, in_=wp[:, :, :])
        ones_s = singles.tile([128, 1], bf16)
        nc.vector.memset(ones_s, 1.0)
        sv_s = singles.tile([K, 1], f32)
        nc.sync.dma_start(out=sv_s, in_=sv[:, :])
        bv_s = singles.tile([K, 1], f32)
        nc.sync.dma_start(out=bv_s, in_=bv[:, :])

        def bcast(st):
            """Stage +1: broadcast 1/r to 19 rows, then t = G * ribc
            (frees the G PSUM banks one tile after they are written)."""
            Gs, idx, b, h0 = st
            ribc = bpool.tile([K, nf], bf16, tag="ribc")
            nc.gpsimd.dma_start(
                out=ribc,
                in_=rid.ap()[idx : idx + 1, :].partition_broadcast(K).squeeze(1),
            )
            t = tpool.tile([K, nf], f32, tag="t")
            for s_ in range(nsub):
                sl = slice(s_ * 512, (s_ + 1) * 512)
                nc.vector.tensor_mul(
                    out=t[:, sl], in0=Gs[s_], in1=ribc[:, sl]
                )
            return (t, b, h0)

        def late(st):
            """Stage +2: u/out, one tile period after t."""
            t, b, h0 = st
            u = opool.tile([K, nf], bf16, tag="u")
            nc.scalar.activation(
                out=u, in_=t, func=AF.Sqrt, bias=bv_s, scale=sv_s
            )
            nc.gpsimd.dma_start(out=out[b, :, h0 : h0 + nf], in_=u)

        pending = []
        pend2 = []
        for b in range(bpc):
            for m in range(nmacro):
                h0 = m * nf
                idx = b * nmacro + m
                xt = xpool.tile([128, 2, nf], f32r, tag="xt")
                # alternate the two HWDGE queues (SP/ACT) so descriptor
                # generation for tile m+1 overlaps tile m's transfer
                eng = nc.sync if idx % 2 == 0 else nc.scalar
                eng.dma_start(
                    out=xt,
                    in_=feat[b, :, h0 : h0 + nf].rearrange(
                        "(j c) n -> c j n", c=128
                    ),
                )

                q1 = qpool.tile([128, nf], bf16, tag="q1")
                nc.vector.tensor_mul(
                    out=q1,
                    in0=xt[:, 0, :].bitcast(f32),
                    in1=xt[:, 0, :].bitcast(f32),
                )
                q2 = qpool.tile([128, nf], bf16, tag="q2")
                nc.scalar.activation(
                    out=q2, in_=xt[:, 1, :].bitcast(f32), func=AF.Square
                )
                qs = qpool.tile([128, nf], bf16, tag="qs")
                nc.vector.tensor_add(out=qs, in0=q1, in1=q2)

                Gs = []
                for s_ in range(nsub):
                    sl = slice(s_ * 512, (s_ + 1) * 512)
                    Gst = gpool.tile([K, 512], f32)
                    nc.tensor.matmul(
                        out=Gst,
                        lhsT=w_s[:, 0, :],
                        rhs=xt[:, 0, sl],
                        start=True,
                        stop=False,
                    )
                    nc.tensor.matmul(
                        out=Gst,
                        lhsT=w_s[:, 1, :],
                        rhs=xt[:, 1, sl],
                        start=False,
                        stop=True,
                    )
                    Gs.append(Gst)
                S = spool.tile([1, nf], f32)
                for s_ in range(nsub):
                    sl = slice(s_ * 512, (s_ + 1) * 512)
                    nc.tensor.matmul(
                        out=S[:, sl],
                        lhsT=ones_s,
                        rhs=qs[:, sl],
                        start=True,
                        stop=True,
                    )
                # r = sqrt(ss) straight out of PSUM ([1,nf]: 1 lane)
                r1 = r1pool.tile([1, nf], bf16, tag="r1")
                nc.scalar.activation(out=r1, in_=S, func=AF.Sqrt)
                # compact so the reciprocal costs nf/128 per lane, then
                # DRAM round-trip to broadcast 1/r across the 19 rows
                cm = cpool.tile([128, ncmp], bf16, tag="cm")
                nc.gpsimd.dma_start(out=cm, in_=r1)
                ic = cpool.tile([128, ncmp], bf16, tag="ic")
                with nc.allow_low_precision(reason="bf16 ok: 2e-2 rel tol"):
                    nc.vector.reciprocal(out=ic, in_=cm)
                nc.gpsimd.dma_start(out=rid.ap()[idx : idx + 1, :], in_=ic)

                if len(pending) == 1:
                    st = bcast(pending.pop(0))
                    pend2.append(st)
                if len(pend2) == 2:
                    late(pend2.pop(0))
                pending.append((Gs, idx, b, h0))
        for st in pending:
            pend2.append(bcast(st))
        for st in pend2:
            late(st)

    return _split_excess_waits(nc) if split_waits else nc


def host_inputs(features, prototypes, distance_scale, bpc=BPC, hw=HW):
    """Build per-core input maps (host-side prep)."""
    pn = prototypes / np.maximum(
        np.sqrt(np.sum(prototypes * prototypes, axis=-1, keepdims=True)), EPS
    )
    s = abs(float(np.asarray(distance_scale).reshape(-1)[0]))
    # wproto[c, j, k] = pn[k, j*128 + c]
    wproto = np.ascontiguousarray(
        pn.T.reshape(2, 128, K).transpose(1, 0, 2)
    ).astype(np.float32)
    svec = np.full((K, 1), -2.0 * s * s, np.float32)
    bvec = np.full((K, 1), 2.0 * s * s, np.float32)

    ncores = features.shape[0] // bpc
    fr = features.reshape(ncores, bpc, C, hw)
    in_maps = []
    for i in range(ncores):
        in_maps.append(
            {
                "features": np.ascontiguousarray(fr[i]),
                "wproto": wproto,
                "svec": svec,
                "bvec": bvec,
            }
        )
    return in_maps


_CACHE = {}


def kernel(features, prototypes, distance_scale):
    from concourse.bass_utils import run_bass_kernel_spmd

    if "nc" not in _CACHE:
        _CACHE["nc"] = build_program()
    nc = _CACHE["nc"]
    in_maps = host_inputs(features, prototypes, distance_scale)
    res = run_bass_kernel_spmd(nc, in_maps, core_ids=list(range(NCORES)))
    out = np.empty((NCORES, BPC, K, H, W), np.float32)
    for i in range(NCORES):
        # device returns u = |s| sqrt(d2) in bf16; negate during upcast
        np.multiply(
            res.results[i]["out"].reshape(BPC, K, H, W).astype(np.float32),
            -1.0,
            out=out[i],
        )
    return out.reshape(B, K, H, W)


# revision 33
# speedup vs baseline: 1.1793x; 1.1793x over previous
"""IsoMaxPlus first-part kernel for Trainium2 (8 NeuronCores, SPMD).

Math (per point n, prototype k):
    c[n,k] = (x_n . p_hat_k) / ||x_n||          (cosine sim)
    out[n,k] = -|s| * sqrt(2 - 2 c[n,k])

The device computes u = |s|*sqrt(2-2c) in bf16; the host negates during
the bf16 -> f32 upcast of the gather (a sqrt's sign cannot be flipped
on-device without a whole extra engine pass).

Per macro-tile of NF=1024 points (per core: 2 of 16 batches):
    DMA  xt [128, 2, NF] f32r                   (C=256 split in 2 chunks)
    DVE  q1 = xt0^2 (bf16)     ACT  q2 = Square(xt1) (bf16)
    DVE  qs = q1 + q2 (bf16)                    (fold 256-chan ssq to 128)
    PE   G[19,NF]  = W.T @ x   (fp32r, 2 passes; W = p_hat)
         S[1,NF]   = ones.T @ qs (bf16, 1 pass; = ||x||^2)
    ACT  r1[1,NF]  = Sqrt(S)                    (PSUM extract, = ||x||)
    DMA  cm[128,8] <- r1                        (compact: 8/lane not 1024)
    DVE  ic = 1/cm                              (= 1/||x||, compact)
    DMA  rid[tile] <- ic                        (DRAM round-trip ...)
    DMA  ribc[19,NF] <- rid (stride-0 x19)      (... broadcast to 19 rows)
    DVE  t = G * ribc                           (= c)
    ACT  u = Sqrt(-2s^2 t + 2s^2) (bf16)        (= |s| sqrt(d2))
    DMA  out <- u

The tail (ribc/t/u/out) of tile m is emitted during tile m+1 so the
compact-chain latency never head-of-line-blocks any engine. PE work is
3 streaming passes/tile -- the engine floor under the ~50% power
throttle observed on these cores (PE never ramps past ~1.2 GHz).
"""

import numpy as np

B, C, H, W = 16, 256, 128, 256
K = 19
NCORES = 8
BPC = B // NCORES          # batches per core
HW = H * W                 # 32768 points per batch
NF = 1024                  # points per macro-tile
EPS = 1e-12


def _split_excess_waits(nc):
    """Walrus limits the sync-wait slots per ISA instruction (TensorTensor
    takes only 1, DMAs 2, ...). Hoist excess waits onto same-engine NoOps
    inserted right before the instruction — engines execute in order, so
    all waits still complete before the instruction runs."""
    import bass_rust
    import concourse.mybir as mybir

    limits = {}
    default_limit = 1
    skip = {"InstEventSemaphore", "InstNoOp", "InstCall",
            "InstUnconditionalBranch", "InstISA", "InstRegisterMove"}
    nseq = 0
    for fn in nc.m.functions:
        for blk in fn.blocks:
            new = []
            for I in blk.instructions:
                tn = type(I).__name__
                si = I.sync_info
                waits = list(si.on_wait) if si else []
                lim = limits.get(tn, default_limit)
                if tn in skip or len(waits) <= lim:
                    new.append(I)
                    continue
                keep = waits[-lim:]
                excess = waits[:-lim]
                for w in excess:
                    nop = mybir.InstNoOp(name=f"{I.name}-w{nseq}", ins=[], outs=[])
                    nseq += 1
                    nop.engine = I.engine
                    nop.sync_info = bass_rust.SyncInfo(on_wait=[w], on_update=[])
                    new.append(nop)
                I.sync_info = bass_rust.SyncInfo(
                    on_wait=keep, on_update=list(si.on_update) if si else []
                )
                new.append(I)
            blk.instructions = new
    return nc


def build_program(bpc=BPC, hw=HW, nf=NF, split_waits=True):
    from contextlib import ExitStack

    import concourse.bass as bass
    import concourse.mybir as mybir
    import concourse.tile as tile

    f32 = mybir.dt.float32
    f32r = mybir.dt.float32r
    bf16 = mybir.dt.bfloat16
    AF = mybir.ActivationFunctionType
    nsub = nf // 512
    nmacro = hw // nf
    ncmp = nf // 128           # compact free size
    ntiles = bpc * nmacro

    nc = bass.Bass()
    # features/wproto declared float32r (same f32 bits) so the fp32r
    # matmuls see fp32r-dtype producers without any conversion step
    feat = nc.declare_dram_parameter("features", [bpc, C, hw], f32r, isOutput=False)
    wp = nc.declare_dram_parameter("wproto", [128, 2, K], f32r, isOutput=False)
    sv = nc.declare_dram_parameter("svec", [K, 1], f32, isOutput=False)
    bv = nc.declare_dram_parameter("bvec", [K, 1], f32, isOutput=False)
    out = nc.declare_dram_parameter("out", [bpc, K, hw], bf16, isOutput=True)
    rid = nc.dram_tensor("ridscratch", (ntiles, nf), bf16, kind="Internal")

    with ExitStack() as ctx:
        tc = ctx.enter_context(tile.TileContext(nc))
        singles = ctx.enter_context(tc.tile_pool(name="singles", bufs=1))
        xpool = ctx.enter_context(tc.tile_pool(name="x", bufs=8))
        qpool = ctx.enter_context(tc.tile_pool(name="q", bufs=6))
        gpool = ctx.enter_context(tc.tile_pool(name="g", bufs=4, space="PSUM"))
        spool = ctx.enter_context(tc.tile_pool(name="s", bufs=2, space="PSUM"))
        cpool = ctx.enter_context(tc.tile_pool(name="c", bufs=10))
        r1pool = ctx.enter_context(tc.tile_pool(name="r1", bufs=6))
        bpool = ctx.enter_context(tc.tile_pool(name="bc", bufs=4))
        tpool = ctx.enter_context(tc.tile_pool(name="t", bufs=4))
        opool = ctx.enter_context(tc.tile_pool(name="o", bufs=4))

        w_s = singles.tile([128, 2, K], f32r)
        nc.sync.dma_start(out=w_s, in_=wp[:, :, :])
        ones_s = singles.tile([128, 1], bf16)
        nc.vector.memset(ones_s, 1.0)
        sv_s = singles.tile([K, 1], f32)
        nc.sync.dma_start(out=sv_s, in_=sv[:, :])
        bv_s = singles.tile([K, 1], f32)
        nc.sync.dma_start(out=bv_s, in_=bv[:, :])

        # Software pipeline, one iteration per tile. Emission order is
        # chosen so every Pool-queue (SWDGE) DMA at the queue head has
        # dependencies that are >= 1 tile old — the in-order queue never
        # head-of-line-blocks on an in-flight engine op. Per iteration m:
        #   ribc(m-1) | u(m-2), out(m-2) | early(m) | t(m-1) | ic/rid(m)
        p1 = []  # tiles awaiting stage+1 (t)      : (Gs, idx, b, h0)
        p2 = []  # tiles awaiting stage+2 (u/out)  : (t, b, h0)

        def stage1_bcast(st):
            Gs, idx, b, h0 = st
            ribc = bpool.tile([K, nf], bf16, tag="ribc")
            nc.gpsimd.dma_start(
                out=ribc,
                in_=rid.ap()[idx : idx + 1, :].partition_broadcast(K).squeeze(1),
            )
            return (Gs, ribc, b, h0)

        def stage2_out(st):
            t, b, h0 = st
            u = opool.tile([K, nf], bf16, tag="u")
            nc.scalar.activation(
                out=u, in_=t, func=AF.Sqrt, bias=bv_s, scale=sv_s
            )
            nc.gpsimd.dma_start(out=out[b, :, h0 : h0 + nf], in_=u)

        def stage1_t(st):
            Gs, ribc, b, h0 = st
            t = tpool.tile([K, nf], f32, tag="t")
            for s_ in range(nsub):
                sl = slice(s_ * 512, (s_ + 1) * 512)
                nc.vector.tensor_mul(out=t[:, sl], in0=Gs[s_], in1=ribc[:, sl])
            return (t, b, h0)

        for b in range(bpc):
            for m in range(nmacro):
                h0 = m * nf
                idx = b * nmacro + m
                # -- queue-head DMAs with aged deps ------------------
                if p1:
                    p1[0] = stage1_bcast(p1[0])
                if len(p2) == 2:
                    stage2_out(p2.pop(0))
                # -- early block of tile m ---------------------------
                xt = xpool.tile([128, 2, nf], f32r, tag="xt")
                nc.sync.dma_start(
                    out=xt,
                    in_=feat[b, :, h0 : h0 + nf].rearrange(
                        "(j c) n -> c j n", c=128
                    ),
                )
                q1 = qpool.tile([128, nf], bf16, tag="q1")
                nc.vector.tensor_mul(
                    out=q1,
                    in0=xt[:, 0, :].bitcast(f32),
                    in1=xt[:, 0, :].bitcast(f32),
                )
                q2 = qpool.tile([128, nf], bf16, tag="q2")
                nc.scalar.activation(
                    out=q2, in_=xt[:, 1, :].bitcast(f32), func=AF.Square
                )
                qs = qpool.tile([128, nf], bf16, tag="qs")
                nc.vector.tensor_add(out=qs, in0=q1, in1=q2)

                Gs = []
                for s_ in range(nsub):
                    sl = slice(s_ * 512, (s_ + 1) * 512)
                    Gst = gpool.tile([K, 512], f32)
                    nc.tensor.matmul(
                        out=Gst,
                        lhsT=w_s[:, 0, :],
                        rhs=xt[:, 0, sl],
                        start=True,
                        stop=False,
                    )
                    nc.tensor.matmul(
                        out=Gst,
                        lhsT=w_s[:, 1, :],
                        rhs=xt[:, 1, sl],
                        start=False,
                        stop=True,
                    )
                    Gs.append(Gst)
                S = spool.tile([1, nf], f32)
                for s_ in range(nsub):
                    sl = slice(s_ * 512, (s_ + 1) * 512)
                    nc.tensor.matmul(
                        out=S[:, sl],
                        lhsT=ones_s,
                        rhs=qs[:, sl],
                        start=True,
                        stop=True,
                    )
                # r = sqrt(ss) straight out of PSUM ([1,nf]: 1 lane)
                r1 = r1pool.tile([1, nf], bf16, tag="r1")
                nc.scalar.activation(out=r1, in_=S, func=AF.Sqrt)
                cm = cpool.tile([128, ncmp], bf16, tag="cm")
                nc.gpsimd.dma_start(out=cm, in_=r1)
                # -- t of tile m-1 (ribc has had a full tile in flight)
                if p1:
                    p2.append(stage1_t(p1.pop(0)))
                # -- compact reciprocal + DRAM stash of tile m -------
                ic = cpool.tile([128, ncmp], bf16, tag="ic")
                with nc.allow_low_precision(reason="bf16 ok: 2e-2 rel tol"):
                    nc.vector.reciprocal(out=ic, in_=cm)
                nc.gpsimd.dma_start(out=rid.ap()[idx : idx + 1, :], in_=ic)
                p1.append((Gs, idx, b, h0))
        while p1 or p2:
            if p1:
                p1[0] = stage1_bcast(p1[0])
                p2.append(stage1_t(p1.pop(0)))
            if p2:
                stage2_out(p2.pop(0))

    return _split_excess_waits(nc) if split_waits else nc


def host_inputs(features, prototypes, distance_scale, bpc=BPC, hw=HW):
    """Build per-core input maps (host-side prep)."""
    pn = prototypes / np.maximum(
        np.sqrt(np.sum(prototypes * prototypes, axis=-1, keepdims=True)), EPS
    )
    s = abs(float(np.asarray(distance_scale).reshape(-1)[0]))
    # wproto[c, j, k] = pn[k, j*128 + c]
    wproto = np.ascontiguousarray(
        pn.T.reshape(2, 128, K).transpose(1, 0, 2)
    ).astype(np.float32)
    svec = np.full((K, 1), -2.0 * s * s, np.float32)
    bvec = np.full((K, 1), 2.0 * s * s, np.float32)

    ncores = features.shape[0] // bpc
    fr = features.reshape(ncores, bpc, C, hw)
    in_maps = []
    for i in range(ncores):
        in_maps.append(
            {
                "features": np.ascontiguousarray(fr[i]),
                "wproto": wproto,
                "svec": svec,
                "bvec": bvec,
            }
        )
    return in_maps


_CACHE = {}


def kernel(features, prototypes, distance_scale):
    from concourse.bass_utils import run_bass_kernel_spmd

    if "nc" not in _CACHE:
        _CACHE["nc"] = build_program()
    nc = _CACHE["nc"]
    in_maps = host_inputs(features, prototypes, distance_scale)
    res = run_bass_kernel_spmd(nc, in_maps, core_ids=list(range(NCORES)))
    out = np.empty((NCORES, BPC, K, H, W), np.float32)
    for i in range(NCORES):
        # device returns u = |s| sqrt(d2) in bf16; negate during upcast
        np.multiply(
            res.results[i]["out"].reshape(BPC, K, H, W).astype(np.float32),
            -1.0,
            out=out[i],
        )
    return out.reshape(B, K, H, W)
